# revision 13
# baseline (speedup 1.0000x reference)
"""CrossAttention Trainium2 kernel (nn_CrossAttention_28544352649420).

Full-input contract: kernel(**inputs) takes the unsharded arrays
  inputA [8,2048,1024] f32, inputB [8,2048,1024] f32,
  maskA [8,2048] f32, maskB [8,2048] f32, W [1024,1024] f32, b [1024] f32
and returns (cvA [8,2048,1024], cvB [8,2048,1024]) matching

  projA  = inputA @ W + b
  scores = projA @ inputB^T, masked_fill(maskA x maskB == 0, -1e9)
  attnA  = softmax(scores, axis=1); attnB = softmax(scores, axis=2)
  cvA    = attnA^T @ inputA;        cvB = attnB @ inputB

Sharding: batch dim across the 8 NeuronCores (data parallel, SPMD —
one batch element per core; every core holds the full W).

Per-core schedule (B=1, La=Lb=2048, Da=Db=1024):
  Phase A: projAT[e,l] = (inputA @ W)^T via PE-transposed inputA strips,
           fp32r matmuls, streamed to a DRAM scratch.
  Phase B: inputBT[e,m] (PE transpose, fp32r) + inputB bf16 copy resident.
  Pass 1 (16 l-strips): S strip = projAT_strip^T @ inputBT (fp32r),
           mask via min-masks, rowmax (exact, per-partition),
           E_B = exp(S - rowmax) bf16 (+ fused denom), per-strip PE
           transpose of E_B -> cvB strip = E_B^T @ inputB_bf / denom.
           Fully-masked scores (both masks) stream to a DRAM scratch.
  Phase C (16 m-chunks): load the masked-score column slab, take the
           exact per-column max (gpsimd partition_all_reduce + reduce),
           E_A = exp(s - colmax) bf16, cvA chunk = E_A^T @ inputA_bf /
           colsum, colsum via ones-vector matmul accumulation.
"""
import sys

sys.path.insert(0, "/opt/trn_rl_repo")

import numpy as np
from contextlib import ExitStack

import concourse.bass as bass
import concourse.tile as tile
from concourse import bacc
from concourse import mybir
from concourse import bass_isa
from concourse.bass_utils import run_bass_kernel_spmd
from concourse.masks import make_identity

F32 = mybir.dt.float32
F32R = mybir.dt.float32r
BF16 = mybir.dt.bfloat16
MIN = mybir.AluOpType.min
MULT = mybir.AluOpType.mult
ADD = mybir.AluOpType.add
SUB = mybir.AluOpType.subtract
EXP = mybir.ActivationFunctionType.Exp
X = mybir.AxisListType.X

B, L, D = 8, 2048, 1024
NS = L // 128  # 16 strips
KC = D // 128  # 8 contraction chunks
BIG = 1.0e30
NEG = -1.0e9
SHIFT = 25.0

_CACHE = {}


def build():
    nc = bacc.Bacc(trn_type="TRN2")

    inputA = nc.declare_dram_parameter("inputA", [L, D], F32, isOutput=False)
    inputB = nc.declare_dram_parameter("inputB", [L, D], F32, isOutput=False)
    maskA = nc.declare_dram_parameter("maskA", [L, 1], F32, isOutput=False)
    maskB = nc.declare_dram_parameter("maskB", [1, L], F32, isOutput=False)
    Wp = nc.declare_dram_parameter("W", [D, D], F32, isOutput=False)
    bp = nc.declare_dram_parameter("b", [D, 1], F32, isOutput=False)
    cvA = nc.declare_dram_parameter("cvA", [L, D], F32, isOutput=True)
    cvB = nc.declare_dram_parameter("cvB", [L, D], F32, isOutput=True)

    projAT_d = nc.dram_tensor("projAT_d", [D, L], F32R)  # (inputA @ W)^T
    TMIN_d = nc.dram_tensor("TMIN_d", [L, L], F32)  # fully-masked scores

    with tile.TileContext(nc) as tc, ExitStack() as ctx:
        glob = ctx.enter_context(tc.tile_pool(name="glob", bufs=1))

        ident = glob.tile([128, 128], F32)
        make_identity(nc, ident)
        ident_bf = glob.tile([128, 128], BF16)
        make_identity(nc, ident_bf)
        ones_bf = glob.tile([128, 1], BF16)
        nc.vector.memset(ones_bf, 1.0)


        b_t = glob.tile([128, KC], F32)
        nc.sync.dma_start(out=b_t, in_=bp[:].rearrange("(c p) o -> p (c o)", p=128))

        # maskA as per-partition column [128,1] per strip; maskB broadcast row
        maA = glob.tile([128, NS], F32)
        nc.sync.dma_start(out=maA, in_=maskA[:].rearrange("(s p) o -> p (s o)", p=128))
        maA_min = glob.tile([128, NS], F32)  # 1 -> +BIG, 0 -> NEG
        nc.vector.tensor_scalar(
            out=maA_min, in0=maA, scalar1=BIG - NEG, scalar2=NEG, op0=MULT, op1=ADD
        )
        mb_t = glob.tile([128, L], F32)
        nc.sync.dma_start(
            out=mb_t, in_=maskB[:].rearrange("o n -> (o n)").partition_broadcast(128)
        )
        MBb = glob.tile([128, L], F32)
        nc.vector.tensor_scalar(
            out=MBb, in0=mb_t, scalar1=BIG - NEG, scalar2=NEG, op0=MULT, op1=ADD
        )


        # ---------------- Phase A: projAT ----------------
        with tc.tile_pool(name="pa", bufs=1) as pa, \
             tc.tile_pool(name="pa_ps", bufs=1, space="PSUM") as pa_ps:
            w_r = pa.tile([128, KC, D], F32R)
            for dc in range(KC):
                w_t = pa.tile([128, D], F32, tag="w_t", bufs=2)
                nc.sync.dma_start(
                    out=w_t, in_=Wp[dc * 128:(dc + 1) * 128, :]
                )
                nc.gpsimd.tensor_copy(out=w_r[:, dc, :], in_=w_t)

            for pair in range(NS // 2):
                at2 = pa.tile([128, KC, 256], F32R, tag="at2", bufs=2)
                for s in range(2):
                    stripA = pa.tile([128, D], F32, tag="stripA", bufs=3)
                    nc.sync.dma_start(
                        out=stripA, in_=inputA[(2 * pair + s) * 128:(2 * pair + s + 1) * 128, :]
                    )
                    for g in range(2):
                        tp = pa_ps.tile([128, 4, 128], F32, tag="tp", bufs=2)
                        for j in range(4):
                            dc = g * 4 + j
                            nc.tensor.transpose(
                                tp[:, j, :], stripA[:, dc * 128:(dc + 1) * 128], ident
                            )
                        nc.scalar.copy(
                            out=at2[:, g * 4:(g + 1) * 4, s * 128:(s + 1) * 128], in_=tp
                        )
                for ec in range(KC):
                    pj = pa_ps.tile([128, 256], F32, tag="pj", bufs=4)
                    for dc in range(KC):
                        nc.tensor.matmul(
                            pj,
                            w_r[:, dc, ec * 128:(ec + 1) * 128],
                            at2[:, dc, :],
                            start=(dc == 0),
                            stop=(dc == KC - 1),
                        )
                    pjs = pa.tile([128, 256], F32R, tag="pjs", bufs=3)
                    nc.vector.tensor_scalar(
                        out=pjs, in0=pj, scalar1=b_t[:, ec:ec + 1], scalar2=None, op0=ADD
                    )
                    nc.sync.dma_start(
                        out=projAT_d[:].rearrange("(c p) l -> p c l", p=128)[
                            :, ec, pair * 256:(pair + 1) * 256
                        ],
                        in_=pjs,
                    )

        # ------- Phase B (inputBT f32r + inputB bf16) and Pass 1 share a pool
        with tc.tile_pool(name="p1", bufs=1) as p1, \
             tc.tile_pool(name="p1_ps", bufs=1, space="PSUM") as p1_ps:
            bt_r = p1.tile([128, KC, L], F32R)  # [e-part, ec, m]
            b_bf = p1.tile([128, NS, D], BF16)  # [m-part, mc, e]
            for i in range(NS):
                stripB = p1.tile([128, D], F32, tag="stripB", bufs=3)
                nc.sync.dma_start(out=stripB, in_=inputB[i * 128:(i + 1) * 128, :])
                nc.gpsimd.tensor_copy(out=b_bf[:, i, :], in_=stripB)
                for g in range(2):
                    tp2 = p1_ps.tile([128, 4, 128], F32, tag="tp", bufs=2)
                    for j in range(4):
                        ec = g * 4 + j
                        nc.tensor.transpose(
                            tp2[:, j, :], stripB[:, ec * 128:(ec + 1) * 128], ident
                        )
                    nc.scalar.copy(
                        out=bt_r[:, g * 4:(g + 1) * 4, i * 128:(i + 1) * 128], in_=tp2
                    )
            for i in range(NS):
                pAT = p1.tile([128, KC, 128], F32R, tag="pAT", bufs=2)
                nc.sync.dma_start(
                    out=pAT,
                    in_=projAT_d[:].rearrange("(c p) l -> p c l", p=128)[
                        :, :, i * 128:(i + 1) * 128
                    ],
                )
                smask = p1.tile([128, L], F32, tag="smask", bufs=2)
                for h in range(2):
                    sps = p1_ps.tile([128, 1024], F32, tag="sps", bufs=2)
                    for nb in range(2):
                        for ec in range(KC):
                            nc.tensor.matmul(
                                sps[:, nb * 512:(nb + 1) * 512],
                                pAT[:, ec, :],
                                bt_r[:, ec, h * 1024 + nb * 512:h * 1024 + (nb + 1) * 512],
                                start=(ec == 0),
                                stop=(ec == KC - 1),
                            )
                    nc.vector.tensor_tensor(
                        out=smask[:, h * 1024:(h + 1) * 1024],
                        in0=sps,
                        in1=MBb[:, h * 1024:(h + 1) * 1024],
                        op=MIN,
                    )
                negrm = p1.tile([128, 1], F32, tag="negrm", bufs=2)
                nc.vector.reduce_max(out=negrm, in_=smask, axis=X, negate=True)
                biasB = p1.tile([128, 1], F32, tag="biasB", bufs=2)
                nc.vector.tensor_tensor(
                    out=biasB, in0=negrm, in1=maA[:, i:i + 1], op=MULT
                )
                eb = p1.tile([128, L], BF16, tag="eb", bufs=2)
                denomB = p1.tile([128, 1], F32, tag="denomB", bufs=2)
                nc.scalar.activation(
                    out=eb, in_=smask, func=EXP,
                    bias=biasB, scale=maA[:, i:i + 1], accum_out=denomB,
                )
                # fully-masked scores (A-mask applied too) -> DRAM for phase C
                tmin = p1.tile([128, L], F32, tag="tmin", bufs=2)
                nc.vector.tensor_scalar_min(tmin, smask, maA_min[:, i:i + 1])
                nc.sync.dma_start(out=TMIN_d[i * 128:(i + 1) * 128, :], in_=tmin)

                # E_B^T tiles and cvB strip
                ebt = p1.tile([128, NS, 128], BF16, tag="ebt", bufs=2)
                for g in range(2):
                    tp3 = p1_ps.tile([128, 8, 128], BF16, tag="tp", bufs=2)
                    for j in range(8):
                        mc = g * 8 + j
                        nc.tensor.transpose(
                            tp3[:, j, :], eb[:, mc * 128:(mc + 1) * 128], ident_bf
                        )
                    nc.scalar.copy(out=ebt[:, g * 8:(g + 1) * 8, :], in_=tp3)
                ups = p1_ps.tile([128, D], F32, tag="ups", bufs=1)
                for nb in range(2):
                    for mc in range(NS):
                        nc.tensor.matmul(
                            ups[:, nb * 512:(nb + 1) * 512],
                            ebt[:, mc, :],
                            b_bf[:, mc, nb * 512:(nb + 1) * 512],
                            start=(mc == 0),
                            stop=(mc == NS - 1),
                        )
                rden = p1.tile([128, 1], F32, tag="rden", bufs=2)
                nc.vector.reciprocal(out=rden, in_=denomB)
                cvb_sb = p1.tile([128, D], F32, tag="cvb_sb", bufs=2)
                nc.vector.tensor_scalar(
                    out=cvb_sb, in0=ups, scalar1=rden, scalar2=None, op0=MULT
                )
                nc.sync.dma_start(out=cvB[i * 128:(i + 1) * 128, :], in_=cvb_sb)

        # ---------------- Phase C: cvA per m-chunk ----------------
        with tc.tile_pool(name="pc", bufs=1) as pc, \
             tc.tile_pool(name="pc_ps", bufs=1, space="PSUM") as pc_ps:
            a_bf = pc.tile([128, NS, D], BF16)  # [l-part, lc, e]
            for i in range(NS):
                stripA2 = pc.tile([128, D], F32, tag="stripA2", bufs=3)
                nc.sync.dma_start(out=stripA2, in_=inputA[i * 128:(i + 1) * 128, :])
                nc.gpsimd.tensor_copy(out=a_bf[:, i, :], in_=stripA2)
            for j in range(NS):
                # tmin column slab: [l-part, lc, m] for 128 columns m
                slab = pc.tile([128, NS, 128], F32, tag="slab", bufs=2)
                nc.sync.dma_start(
                    out=slab,
                    in_=TMIN_d[:, j * 128:(j + 1) * 128].rearrange(
                        "(c p) m -> p c m", p=128
                    ),
                )
                # exact per-column max: allreduce over partitions, then over lc
                colr = pc.tile([128, NS, 128], F32, tag="colr", bufs=2)
                nc.gpsimd.partition_all_reduce(
                    colr.rearrange("p a b -> p (a b)"),
                    slab.rearrange("p a b -> p (a b)"),
                    channels=128,
                    reduce_op=bass_isa.ReduceOp.max,
                )
                cm = pc.tile([128, 128], F32, tag="cm", bufs=2)
                nc.vector.reduce_max(
                    out=cm, in_=colr.rearrange("p a b -> p b a"), axis=X
                )
                # E_A tiles = exp(tmin - colmax), bf16
                for lc in range(NS):
                    nc.vector.tensor_tensor(
                        out=slab[:, lc, :], in0=slab[:, lc, :], in1=cm, op=SUB
                    )
                ea_t = pc.tile([128, NS, 128], BF16, tag="ea_t", bufs=2)
                nc.scalar.activation(out=ea_t, in_=slab, func=EXP)
                aps = pc_ps.tile([128, D], F32, tag="aps", bufs=2)
                csum = pc_ps.tile([128, 1], F32, tag="csum", bufs=2)
                for lc in range(NS):
                    nc.tensor.matmul(
                        csum, ea_t[:, lc, :], ones_bf,
                        start=(lc == 0), stop=(lc == NS - 1),
                    )
                for nb in range(2):
                    for lc in range(NS):
                        nc.tensor.matmul(
                            aps[:, nb * 512:(nb + 1) * 512],
                            ea_t[:, lc, :],
                            a_bf[:, lc, nb * 512:(nb + 1) * 512],
                            start=(lc == 0),
                            stop=(lc == NS - 1),
                        )
                rcs = pc.tile([128, 1], F32, tag="rcs", bufs=2)
                nc.vector.reciprocal(out=rcs, in_=csum)
                cva_sb = pc.tile([128, D], F32, tag="cva_sb", bufs=2)
                nc.vector.tensor_scalar(
                    out=cva_sb, in0=aps, scalar1=rcs, scalar2=None, op0=MULT
                )
                nc.sync.dma_start(out=cvA[j * 128:(j + 1) * 128, :], in_=cva_sb)

    if not nc.is_finalized():
        nc.finalize()
    return nc


def run(inputs, trace=False, trace_kwargs=None):
    if "nc" not in _CACHE:
        _CACHE["nc"] = build()
    nc = _CACHE["nc"]
    in_maps = []
    for i in range(B):
        in_maps.append({
            "inputA": np.ascontiguousarray(inputs["inputA"][i], dtype=np.float32),
            "inputB": np.ascontiguousarray(inputs["inputB"][i], dtype=np.float32),
            "maskA": np.ascontiguousarray(
                inputs["maskA"][i], dtype=np.float32).reshape(L, 1),
            "maskB": np.ascontiguousarray(
                inputs["maskB"][i], dtype=np.float32).reshape(1, L),
            "W": np.ascontiguousarray(inputs["W"], dtype=np.float32),
            "b": np.ascontiguousarray(inputs["b"], dtype=np.float32).reshape(D, 1),
        })
    try:
        res = run_bass_kernel_spmd(
            nc, in_maps, core_ids=list(range(B)), trace=trace,
            **(trace_kwargs or {}),
        )
    except ModuleNotFoundError:
        res = run_bass_kernel_spmd(nc, in_maps, core_ids=list(range(B)), trace=False)
    cva = np.stack([res.results[i]["cvA"] for i in range(B)]).astype(np.float32)
    cvb = np.stack([res.results[i]["cvB"] for i in range(B)]).astype(np.float32)
    return (cva, cvb), res


def kernel(**inputs):
    (cva, cvb), _ = run(inputs, trace=False)
    return cva, cvb
